# revision 13
# baseline (speedup 1.0000x reference)
"""Trainium2 Bass kernel for nn_BinaryPathEncoder (v3 — tree table, no gather).

Math: output row for position p is ones(256) pushed through a chain of
matrices P0/P1 chosen by the bits of p (LSB-first, topmost set bit dropped).
All distinct bit-paths form a complete binary tree with 2^17-1 nodes; level
k+1 of the tree is [P0 @ V_k, P1 @ V_k] so the whole tree costs ~17 GFLOP.
Every output row is then a lookup into the tree table.

Sharding: tree nodes are assigned to core p mod 8; children of a core's
node stay on that core, so each core's subtree is self-contained with zero
cross-core communication.  The host computes the tiny levels 0..9 (1023
rows, 0.8% of the table) and hands each core its 64 level-9 seed vectors;
the device builds levels 10..16 (16256 rows/core, 99% of the FLOPs) as
fp32r matmuls in column layout [dim, nodes]:

  child half (b,i) = sum_j P_b^T[128j:, 128i:]^T @ V[j]     (PSUM f32)

fp32r runs the PE at bf16 rate for moving dims >= 256 and keeps the chain
at ~1e-3 rel err (vs the 2e-2 envelope); a single bf16 rounding happens
only at emission.  No on-device gather: the full per-core table streams
out (~11.5 MB/core) and the host does the final index lookup.

Emission paths (so no single engine trails the PE):
  levels 10..13 -> DVE bf16 copy of the chain tile, SP HWDGE ring
  levels 14..15 -> raw f32 DMA of the chain tile itself, ACT HWDGE ring
  level 16      -> ACT bf16 drains straight from PSUM, SP HWDGE ring
Chain drains (PSUM -> f32r V tiles) alternate DVE/ACT per phase because a
lone DVE (0.96 GHz) cannot match the PE's 1.2 GHz-equivalent column rate.
"""

import numpy as np
import ml_dtypes

DIM = 256
NCORES = 8
DEV_LV0 = 10                       # first device-built level
L_MAX = 16                         # deepest tree level (positions < 2^17)
SEED_C = 1 << (DEV_LV0 - 4)        # 64 level-(DEV_LV0-1) cols per core
VMAX = 1 << (L_MAX - 4)            # widest chain level (level 15: 4096)
BF_COLS = 128 + 256 + 512 + 1024 + 8192   # levels 10..13 + 16
F32_COLS = 2048 + 4096                    # levels 14..15


# ---------------------------------------------------------------------------
# device program
# ---------------------------------------------------------------------------

def build_program():
    import concourse.tile as tile
    import concourse.mybir as mybir
    from concourse import bacc

    f32 = mybir.dt.float32
    f32r = mybir.dt.float32r
    bf16 = mybir.dt.bfloat16
    COPY = mybir.ActivationFunctionType.Copy

    nc = bacc.Bacc("TRN2", target_bir_lowering=False, debug=False,
                   num_devices=NCORES)

    primsT = nc.dram_tensor("primsT", [2, DIM, DIM], f32,
                            kind="ExternalInput").ap()
    seeds = nc.dram_tensor("seeds", [2, 128, SEED_C], f32,
                           kind="ExternalInput").ap()
    out_bf = nc.dram_tensor("out_bf", [2, 128, BF_COLS], bf16,
                            kind="ExternalOutput").ap()
    out_f32 = nc.dram_tensor("out_f32", [2, 128, F32_COLS], f32,
                             kind="ExternalOutput").ap()

    from contextlib import ExitStack
    with tile.TileContext(nc) as tc:
        with ExitStack() as ctx:
            cpool = ctx.enter_context(tc.tile_pool(name="consts", bufs=1))
            vpool = ctx.enter_context(tc.tile_pool(name="vbufs", bufs=2))
            epool = ctx.enter_context(tc.tile_pool(name="emit", bufs=2))
            hpool = ctx.enter_context(tc.tile_pool(name="emith", bufs=1))
            ppool = ctx.enter_context(tc.tile_pool(name="pc", bufs=4,
                                                   space="PSUM"))

            # ---- constants -----------------------------------------------
            # fp32r operands must be produced as f32r (the writing engine
            # applies the rounding).  One batched DMA per input, split
            # across the two HWDGE rings, then one convert-copy each.
            sraw = cpool.tile([128, 2 * SEED_C], f32, tag="sraw", name="sraw")
            nc.sync.dma_start(sraw[:].rearrange("p (j m) -> p j m", j=2),
                              seeds.rearrange("j p m -> p j m"))
            praw = cpool.tile([128, 4 * DIM], f32, tag="praw", name="praw")
            prsrc = primsT.rearrange("b (j p) d -> p (b j) d", p=128)
            prdst = praw[:].rearrange("p (g d) -> p g d", g=4)
            nc.scalar.dma_start(prdst[:, :2], prsrc[:, :2])
            nc.sync.dma_start(prdst[:, 2:], prsrc[:, 2:])
            vs = cpool.tile([128, 2 * SEED_C], f32r, tag="vs", name="vs")
            nc.scalar.activation(vs[:], sraw[:], COPY)
            pt = cpool.tile([128, 4 * DIM], f32r, tag="pt", name="pt")
            nc.vector.tensor_copy(pt[:, :2 * DIM], praw[:, :2 * DIM])
            nc.scalar.activation(pt[:, 2 * DIM:], praw[:, 2 * DIM:], COPY)
            pT = [[pt[:, (2 * b + j) * DIM:(2 * b + j + 1) * DIM]
                   for j in range(2)] for b in range(2)]
            V = [vs[:, SEED_C * j:SEED_C * (j + 1)] for j in range(2)]

            # chain drains alternate DVE / ACT so neither trails the PE
            flip = [0]

            def chain_drain(dst_ap, src_ap):
                if flip[0] % 2 == 0:
                    nc.vector.tensor_copy(dst_ap, src_ap)
                else:
                    nc.scalar.activation(dst_ap, src_ap, COPY)
                flip[0] += 1

            # ---- levels DEV_LV0..L_MAX -----------------------------------
            c = SEED_C
            off_bf = 0
            off_f = 0
            for k in range(DEV_LV0, L_MAX + 1):
                cc = 2 * c
                last = k == L_MAX
                small = k <= 13
                if not last:
                    newV = [vpool.tile([128, VMAX], f32r, tag=f"V{j}",
                                       name=f"V{k}_{j}")[:, :cc]
                            for j in range(2)]
                if small:
                    emit = [epool.tile([128, 1024], bf16, tag=f"E{j}",
                                       name=f"E{k}_{j}")[:, :cc]
                            for j in range(2)]
                elif last:
                    emit = [hpool.tile([128, cc], bf16, tag=f"H{j}",
                                       name=f"H{j}")
                            for j in range(2)]
                # s0-major: columns are consumed in the same order the
                # previous level produced them, so level transitions never
                # wait on trailing drains.
                for s0 in range(0, c, 1024):
                    w = min(1024, c - s0)
                    for b in range(2):
                        for i in range(2):
                            wslice = slice(128 * i, 128 * (i + 1))
                            ps = ppool.tile([128, 1024], f32, tag="PC",
                                            name="ps")[:, :w]
                            for q0 in range(0, w, 512):
                                qw = min(512, w - q0)
                                nc.tensor.matmul(
                                    ps[:, q0:q0 + qw],
                                    pT[b][0][:, wslice],
                                    V[0][:, s0 + q0:s0 + q0 + qw],
                                    start=True, stop=False)
                                nc.tensor.matmul(
                                    ps[:, q0:q0 + qw],
                                    pT[b][1][:, wslice],
                                    V[1][:, s0 + q0:s0 + q0 + qw],
                                    start=False, stop=True)
                            d0 = b * c + s0
                            chain_drain((emit[i] if last else newV[i])
                                        [:, d0:d0 + w], ps[:, :w])
                            if last and (s0 + w) % 2048 == 0:
                                # flush each 2048-col chunk as it completes
                                f0 = b * c + s0 + w - 2048
                                nc.sync.dma_start(
                                    out_bf[i, :, off_bf + f0:
                                           off_bf + f0 + 2048],
                                    emit[i][:, f0:f0 + 2048])
                if small:
                    for i in range(2):
                        nc.vector.tensor_copy(emit[i][:, :cc],
                                              newV[i][:, :cc].bitcast(f32))
                        nc.sync.dma_start(out_bf[i, :, off_bf:off_bf + cc],
                                          emit[i][:, :cc])
                    off_bf += cc
                elif not last:
                    for i in range(2):
                        nc.scalar.dma_start(out_f32[i, :, off_f:off_f + cc],
                                            newV[i][:, :cc].bitcast(f32))
                    off_f += cc
                else:
                    off_bf += cc
                if not last:
                    V = newV
                c = cc

    nc.compile()
    return nc


# ---------------------------------------------------------------------------
# host side
# ---------------------------------------------------------------------------

def _host_levels(primitives, identity):
    """Table rows for p < 2^DEV_LV0 (levels 0..DEV_LV0-1) in fp32."""
    T = np.zeros((1 << DEV_LV0, DIM), np.float32)
    T[0] = identity[0]
    T[1] = identity[0]
    for k in range(1, DEV_LV0):
        prev = T[1 << (k - 1):1 << k]
        half = 1 << (k - 1)
        T[1 << k:(1 << k) + half] = prev @ primitives[0].T
        T[(1 << k) + half:1 << (k + 1)] = prev @ primitives[1].T
    return T


_PROGRAM_CACHE = {}


def _run(unique, primitives, identity, **run_kwargs):
    from concourse.bass_utils import run_bass_kernel_spmd

    unique = np.asarray(unique)
    primitives = np.ascontiguousarray(np.asarray(primitives, np.float32))
    identity = np.ascontiguousarray(np.asarray(identity, np.float32))

    if "prog" not in _PROGRAM_CACHE:
        _PROGRAM_CACHE["prog"] = build_program()
    nc = _PROGRAM_CACHE["prog"]

    Th = _host_levels(primitives, identity)          # rows p < 1024
    primsT = np.ascontiguousarray(primitives.transpose(0, 2, 1))

    in_maps = []
    for core in range(NCORES):
        sc = Th[(1 << (DEV_LV0 - 1)) + core:1 << DEV_LV0:NCORES]  # [64, 256]
        seeds = np.ascontiguousarray(sc.T.reshape(2, 128, SEED_C))
        in_maps.append({"primsT": primsT, "seeds": seeds})

    res = run_bass_kernel_spmd(nc, in_maps, core_ids=list(range(NCORES)),
                               **run_kwargs)

    # assemble the full table, then one bulk lookup
    Tfull = np.zeros((1 << (L_MAX + 1), DIM), np.float32)
    Tfull[:1 << DEV_LV0] = Th
    for core in range(NCORES):
        r = res.results[core]
        bf = np.asarray(r["out_bf"])
        if bf.dtype != ml_dtypes.bfloat16:
            bf = bf.view(ml_dtypes.bfloat16)
        bf = bf.astype(np.float32).reshape(2 * 128, BF_COLS)
        f3 = np.asarray(r["out_f32"]).reshape(2 * 128, F32_COLS)
        off_bf = 0
        off_f = 0
        for k in range(DEV_LV0, L_MAX + 1):
            cc = 1 << (k - 3)
            if k <= 13 or k == L_MAX:
                vals = bf[:, off_bf:off_bf + cc].T
                off_bf += cc
            else:
                vals = f3[:, off_f:off_f + cc].T
                off_f += cc
            Tfull[(1 << k) + core:1 << (k + 1):NCORES] = vals
    outv = Tfull[unique.astype(np.int64)]
    return outv, res


def kernel(unique, primitives, identity):
    out, _ = _run(unique, primitives, identity)
    return out


if __name__ == "__main__":
    # tiny smoke run (full shapes) — prefer test.py for the real check
    rng = np.random.default_rng(0)
    u = rng.integers(0, 1 << 17, size=131072).astype(np.int32)
    prims = rng.standard_normal((2, DIM, DIM)).astype(np.float32)
    ones = np.ones((1, DIM), np.float32)
    out = kernel(u, prims, ones)
    print("kernel output", out.shape, out.dtype)
